# revision 31
# baseline (speedup 1.0000x reference)
"""Trainium2 Bass kernel for Luong bilinear attention.

  out = softmax((q @ w) @ k^T) @ v      q:[B,Lq,Din] k,v:[B,Lk,Dout] w:[Din,Dout]

Sharding: 8 cores = 4 batches x 2 halves of Lq (data-parallel over batch,
sequence-parallel over Lq). k, v are replicated across the 2 cores of a batch.

Per-core layout strategy: scores are computed transposed, sT[k, q], so the
softmax denominator and the attention*V product are both plain matmuls with
k as the contraction (partition) dim:
    wqT[o, q] = w[i,o]^T . qT[i, q]          (PE, fp16)
    sT[k, q]  = kT[o, k]^T . wqT[o, q]       (PE, fp16, f32 PSUM)
    p[k, q]   = exp(sT)                      (ScalarE, f32 -> bf16)
    acc[q, 0:257] = p^T . [v | ones]         (PE; col 256 = softmax denom)
    out[q, o] = acc[:, 0:256] * (1/acc[:, 256])   (DVE)
exp() is applied without max-subtraction: scores ~ N(0, 12.8), |s| < ~70,
exp stays comfortably inside f32/bf16 range, and softmax is shift-invariant.

q and k are fed pre-transposed from the host (qT, kT): marshalling done while
sharding, since fp32 DMA-transpose is not supported on TRN2.

Profile (NTFF, 2.4GHz): the matmul stream is fully dense at the hardware
issue rates (216ns per N=512 fp16 score matmul, 111ns per N=257 bf16 av
matmul), i.e. at the bf16 PE roofline (~113us of irreducible streaming for
the 4.5 GMAC per core); total is roofline + ~8us framework preamble/loads
+ ~3-5us output-flush/exit epilogue. fp8 paths fail the accuracy budget
(near-one-hot softmax flips argmaxes under e4m3/e3m4 score noise), and
hi/lo-compensated DoubleRow is slower than fp16.
"""

import numpy as np

B, LQ, LK, DIN, DOUT = 4, 4096, 4096, 256, 256
N_CORES = 8
QS = LQ // (N_CORES // B)  # 2048 queries per core
QC = 512                   # q-chunk (matmul free dim)
NQC = QS // QC             # 4 chunks
NKT = LK // 128            # 32 k tiles
VN = DOUT + 1              # v plus ones column

_prog_cache: dict = {}


def build_program(repeat: int = 1):
    """Build the (SPMD-identical) per-core Bass program."""
    if repeat in _prog_cache:
        return _prog_cache[repeat]
    from contextlib import ExitStack

    import concourse.bacc as bacc
    import concourse.mybir as mybir
    import concourse.tile as tile

    BF16 = mybir.dt.bfloat16
    FP16 = mybir.dt.float16
    F32 = mybir.dt.float32
    EXP = mybir.ActivationFunctionType.Exp

    nc = bacc.Bacc(
        "TRN2", target_bir_lowering=False, debug=False, num_devices=N_CORES
    )
    # qT host-marshalled plane-major: a "(t p) q" rearranged load lowers to
    # two chained DMA instructions (2nd gated on 1st completion) and lands
    # ~2us later; plane-major is one clean 2D DMA
    qT_d = nc.dram_tensor("qT", [128, 2, QS], FP16, kind="ExternalInput")
    kT_d = nc.dram_tensor("kT", [DOUT, LK], FP16, kind="ExternalInput")
    v_d = nc.dram_tensor("v", [LK, DOUT], BF16, kind="ExternalInput")
    w_d = nc.dram_tensor("w", [DIN, DOUT], FP16, kind="ExternalInput")
    o_d = nc.dram_tensor("o", [QS, DOUT], F32, kind="ExternalOutput")

    with tile.TileContext(nc) as tc, ExitStack() as ctx:
        persist = ctx.enter_context(tc.tile_pool(name="persist", bufs=1))
        pexp = ctx.enter_context(tc.tile_pool(name="pexp", bufs=2))
        ps_pool = ctx.enter_context(
            tc.tile_pool(name="ps", bufs=3, space="PSUM")
        )
        po_pool = ctx.enter_context(
            tc.tile_pool(name="po", bufs=2, space="PSUM")
        )
        outp = ctx.enter_context(tc.tile_pool(name="outp", bufs=4))

        KPC = 8  # kT load pieces
        NWARM = 8  # PE warm-up matmuls on a DVE-memset tile (no DMA dep):
        # the array is busy from right after the entry barrier (~7.4us, vs
        # ~9.5us when gated on the w load) so HAM reaches full clock before
        # the real matmuls arrive
        for _ in range(repeat):
            wu = persist.tile([128, 2, DOUT], FP16, tag="wu")
            nc.vector.memset(wu[:], 0.0)
            wps = ps_pool.tile([128, 2, QC], F32, tag="ps")
            for _i in range(NWARM):
                nc.tensor.matmul(
                    wps[:, 0, :], wu[:, 0, 0:128], wu[:, :, :],
                    start=True, stop=True,
                )

            # ---- loads: inputs arrive pre-cast (fp16 / bf16) from the
            # ---- host marshalling step, so DMAs feed compute tiles directly
            w_bf = persist.tile([128, 2, DOUT], FP16, tag="w_bf")
            nc.sync.dma_start(w_bf[:], w_d.ap().rearrange("(t p) o -> p t o", p=128))

            qT_bf = persist.tile([128, 2, QS], FP16, tag="qT_bf")
            nc.sync.dma_start(qT_bf[:], qT_d.ap())

            kT_r = kT_d.ap().rearrange("(t p) k -> p t k", p=128)
            kT_bf = persist.tile([128, 2, LK], FP16, tag="kT_bf")
            v_r = v_d.ap().rearrange("(t p) o -> p t o", p=128)
            v_bf = persist.tile([128, NKT, VN], BF16, tag="v_bf")
            nc.vector.memset(v_bf[:, :, DOUT : DOUT + 1], 1.0)
            KP = LK // KPC

            def load_kp(kp, act_cast=False):
                sl = slice(kp * KP, (kp + 1) * KP)
                nc.sync.dma_start(kT_bf[:, :, sl], kT_r[:, :, sl])

            def load_v(vh, nvh):
                sl = slice(vh * (NKT // nvh), (vh + 1) * (NKT // nvh))
                nc.sync.dma_start(v_bf[:, sl, 0:DOUT], v_r[:, sl, :])

            # interleave: kT quarters feed scores(0) progressively; v quarters
            # arrive early enough for av(0) to fill score-phase PE gaps;
            # wq phase emitted mid-sequence so its PSUM->SBUF copies win
            # DVE priority over the v casts
            load_kp(0, act_cast=True)
            load_kp(1, act_cast=True)
            load_kp(2, act_cast=True)
            load_kp(3, act_cast=True)

            # ---- wqT[o, q] = w^T . qT ----
            wq_bf = persist.tile([128, 2, QS], FP16, tag="wq_bf")
            for qc2 in range(NQC // 2):
                for ot in range(2):
                    ps = ps_pool.tile([128, 2, QC], F32, tag="ps")
                    for j in range(2):
                        qc = qc2 * 2 + j
                        for it in range(2):
                            nc.tensor.matmul(
                                ps[:, j, :],
                                w_bf[:, it, ot * 128 : (ot + 1) * 128],
                                qT_bf[:, it, qc * QC : (qc + 1) * QC],
                                start=(it == 0),
                                stop=(it == 1),
                            )
                    nc.vector.tensor_copy(
                        wq_bf[:, ot, qc2 * 2 * QC : (qc2 + 1) * 2 * QC],
                        ps[:, :, :],
                    )

            load_v(0, 4)
            load_kp(4)
            load_v(1, 4)
            load_kp(5)
            load_kp(6)
            load_v(2, 4)
            load_kp(7)
            load_v(3, 4)

            # ---- main loop: emit scores(qc+1) before AV(qc) so ScalarE's
            # ---- exp always has PE runway to hide behind
            def scores(qc):
                p_all = pexp.tile([128, NKT, QC], BF16, tag="p_all")
                for ktg in range(NKT // 2):
                    ps = ps_pool.tile([128, 2, QC], F32, tag="ps")
                    for j in range(2):
                        kt = ktg * 2 + j
                        for it in range(2):
                            nc.tensor.matmul(
                                ps[:, j, :],
                                kT_bf[:, it, kt * 128 : (kt + 1) * 128],
                                wq_bf[:, it, qc * QC : (qc + 1) * QC],
                                start=(it == 0),
                                stop=(it == 1),
                            )
                    nc.scalar.activation(
                        p_all[:, ktg * 2 : (ktg + 1) * 2, :], ps[:, :, :], EXP
                    )
                return p_all

            def av(qc, p_all):
                for qt in range(QC // 128):
                    po = po_pool.tile([128, VN], F32, tag="po")
                    for kt in range(NKT):
                        nc.tensor.matmul(
                            po[:],
                            p_all[:, kt, qt * 128 : (qt + 1) * 128],
                            v_bf[:, kt, :],
                            start=(kt == 0),
                            stop=(kt == NKT - 1),
                        )
                    rec = outp.tile([128, 1], F32, tag="rec")
                    nc.vector.reciprocal(rec[:], po[:, DOUT : DOUT + 1])
                    o_sb = outp.tile([128, DOUT], F32, tag="o_sb")
                    nc.vector.tensor_scalar_mul(o_sb[:], po[:, 0:DOUT], rec[:])
                    r0 = (qc * (QC // 128) + qt) * 128
                    nc.sync.dma_start(o_d.ap()[r0 : r0 + 128, :], o_sb[:])

            p_prev = scores(0)
            for qc in range(1, NQC):
                p_cur = scores(qc)
                av(qc - 1, p_prev)
                p_prev = p_cur
            av(NQC - 1, p_prev)

    nc.compile()
    _prog_cache[repeat] = nc
    return nc


def make_in_maps(q, k, v, w):
    """Shard + marshal full inputs into per-core input maps.

    Marshalling includes the transpose of q/k and the rounding to the
    kernel's compute dtypes (fp16 score path, bf16 values) -- the device
    kernel consumes these layouts directly.
    """
    import ml_dtypes

    q = np.asarray(q, dtype=np.float32)
    k = np.asarray(k, dtype=np.float32)
    v = np.asarray(v, dtype=np.float32)
    w16 = np.ascontiguousarray(np.asarray(w, dtype=np.float32)).astype(np.float16)
    kT = [np.ascontiguousarray(k[b].T).astype(np.float16) for b in range(B)]
    vb = [np.ascontiguousarray(v[b]).astype(ml_dtypes.bfloat16) for b in range(B)]
    in_maps = []
    for c in range(N_CORES):
        b, h = divmod(c, N_CORES // B)
        in_maps.append(
            {
                "qT": q[b, h * QS : (h + 1) * QS, :]
                .T.astype(np.float16)
                .reshape(2, 128, QS)
                .transpose(1, 0, 2)
                .copy(),
                "kT": kT[b],
                "v": vb[b],
                "w": w16,
            }
        )
    return in_maps


def kernel(q, v, k, w):
    from concourse import bass_utils

    nc = build_program()
    in_maps = make_in_maps(q, k, v, w)
    res = bass_utils.run_bass_kernel_spmd(nc, in_maps, core_ids=list(range(N_CORES)))
    out = np.empty((B, LQ, DOUT), dtype=np.float32)
    for c in range(N_CORES):
        b, h = divmod(c, N_CORES // B)
        out[b, h * QS : (h + 1) * QS, :] = res.results[c]["o"]
    return out


# revision 32
# speedup vs baseline: 1.1973x; 1.1973x over previous
"""Trainium2 Bass kernel for Luong bilinear attention.

  out = softmax((q @ w) @ k^T) @ v      q:[B,Lq,Din] k,v:[B,Lk,Dout] w:[Din,Dout]

Sharding: 8 cores = 4 batches x 2 halves of Lq (data-parallel over batch,
sequence-parallel over Lq). k, v are replicated across the 2 cores of a batch.

Per-core layout strategy: scores are computed transposed, sT[k, q], so the
softmax denominator and the attention*V product are both plain matmuls with
k as the contraction (partition) dim:
    wqT[o, q] = w[i,o]^T . qT[i, q]          (PE, fp16)
    sT[k, q]  = kT[o, k]^T . wqT[o, q]       (PE, fp16, f32 PSUM)
    p[k, q]   = exp(sT)                      (ScalarE, f32 -> bf16)
    acc[q, 0:257] = p^T . [v | ones]         (PE; col 256 = softmax denom)
    out[q, o] = acc[:, 0:256] * (1/acc[:, 256])   (DVE)
exp() is applied without max-subtraction: scores ~ N(0, 12.8), |s| < ~70,
exp stays comfortably inside f32/bf16 range, and softmax is shift-invariant.

q and k are fed pre-transposed from the host (qT, kT): marshalling done while
sharding, since fp32 DMA-transpose is not supported on TRN2.

Profile (NTFF, 2.4GHz): the matmul stream is fully dense at the hardware
issue rates (216ns per N=512 fp16 score matmul, 111ns per N=257 bf16 av
matmul), i.e. at the bf16 PE roofline (~113us of irreducible streaming for
the 4.5 GMAC per core); total is roofline + ~8us framework preamble/loads
+ ~3-5us output-flush/exit epilogue. fp8 paths fail the accuracy budget
(near-one-hot softmax flips argmaxes under e4m3/e3m4 score noise), and
hi/lo-compensated DoubleRow is slower than fp16.
"""

import numpy as np

B, LQ, LK, DIN, DOUT = 4, 4096, 4096, 256, 256
N_CORES = 8
QS = LQ // (N_CORES // B)  # 2048 queries per core
QC = 512                   # q-chunk (matmul free dim)
NQC = QS // QC             # 4 chunks
NKT = LK // 128            # 32 k tiles
VN = DOUT + 1              # v plus ones column

_prog_cache: dict = {}


def build_program(repeat: int = 1):
    """Build the (SPMD-identical) per-core Bass program."""
    if repeat in _prog_cache:
        return _prog_cache[repeat]
    from contextlib import ExitStack

    import concourse.bacc as bacc
    import concourse.mybir as mybir
    import concourse.tile as tile

    BF16 = mybir.dt.bfloat16
    FP16 = mybir.dt.float16
    F32 = mybir.dt.float32
    EXP = mybir.ActivationFunctionType.Exp

    nc = bacc.Bacc(
        "TRN2", target_bir_lowering=False, debug=False, num_devices=N_CORES
    )
    qT_d = nc.dram_tensor("qT", [DIN, QS], FP16, kind="ExternalInput")
    kT_d = nc.dram_tensor("kT", [DOUT, LK], FP16, kind="ExternalInput")
    v_d = nc.dram_tensor("v", [LK, DOUT], BF16, kind="ExternalInput")
    w_d = nc.dram_tensor("w", [DIN, DOUT], FP16, kind="ExternalInput")
    o_d = nc.dram_tensor("o", [QS, DOUT], F32, kind="ExternalOutput")

    with tile.TileContext(nc) as tc, ExitStack() as ctx:
        persist = ctx.enter_context(tc.tile_pool(name="persist", bufs=1))
        pexp = ctx.enter_context(tc.tile_pool(name="pexp", bufs=2))
        ps_pool = ctx.enter_context(
            tc.tile_pool(name="ps", bufs=3, space="PSUM")
        )
        po_pool = ctx.enter_context(
            tc.tile_pool(name="po", bufs=2, space="PSUM")
        )
        outp = ctx.enter_context(tc.tile_pool(name="outp", bufs=4))

        KPC = 8  # kT load pieces
        NWARM = 8  # PE warm-up matmuls on the (tiny, first-loaded) w tile:
        # keep the array busy from the entry barrier so HAM reaches full
        # clock before the real matmuls arrive
        for _ in range(repeat):
            # ---- loads: inputs arrive pre-cast (fp16 / bf16) from the
            # ---- host marshalling step, so DMAs feed compute tiles directly
            w_bf = persist.tile([128, 2, DOUT], FP16, tag="w_bf")
            nc.sync.dma_start(w_bf[:], w_d.ap().rearrange("(t p) o -> p t o", p=128))
            wps = ps_pool.tile([128, 2, QC], F32, tag="ps")
            for _i in range(NWARM):
                nc.tensor.matmul(
                    wps[:, 0, :], w_bf[:, 0, 0:128], w_bf[:, :, :],
                    start=True, stop=True,
                )

            qT_r = qT_d.ap().rearrange("(t p) q -> p t q", p=128)
            qT_bf = persist.tile([128, 2, QS], FP16, tag="qT_bf")
            nc.sync.dma_start(qT_bf[:], qT_r[:])

            kT_r = kT_d.ap().rearrange("(t p) k -> p t k", p=128)
            kT_bf = persist.tile([128, 2, LK], FP16, tag="kT_bf")
            v_r = v_d.ap().rearrange("(t p) o -> p t o", p=128)
            v_bf = persist.tile([128, NKT, VN], BF16, tag="v_bf")
            nc.vector.memset(v_bf[:, :, DOUT : DOUT + 1], 1.0)
            KP = LK // KPC

            def load_kp(kp, act_cast=False):
                sl = slice(kp * KP, (kp + 1) * KP)
                nc.sync.dma_start(kT_bf[:, :, sl], kT_r[:, :, sl])

            def load_v(vh, nvh):
                sl = slice(vh * (NKT // nvh), (vh + 1) * (NKT // nvh))
                nc.sync.dma_start(v_bf[:, sl, 0:DOUT], v_r[:, sl, :])

            # interleave: kT quarters feed scores(0) progressively; v quarters
            # arrive early enough for av(0) to fill score-phase PE gaps;
            # wq phase emitted mid-sequence so its PSUM->SBUF copies win
            # DVE priority over the v casts
            load_kp(0, act_cast=True)
            load_kp(1, act_cast=True)
            load_kp(2, act_cast=True)
            load_kp(3, act_cast=True)

            # ---- wqT[o, q] = w^T . qT ----
            wq_bf = persist.tile([128, 2, QS], FP16, tag="wq_bf")
            for qc2 in range(NQC // 2):
                for ot in range(2):
                    ps = ps_pool.tile([128, 2, QC], F32, tag="ps")
                    for j in range(2):
                        qc = qc2 * 2 + j
                        for it in range(2):
                            nc.tensor.matmul(
                                ps[:, j, :],
                                w_bf[:, it, ot * 128 : (ot + 1) * 128],
                                qT_bf[:, it, qc * QC : (qc + 1) * QC],
                                start=(it == 0),
                                stop=(it == 1),
                            )
                    nc.vector.tensor_copy(
                        wq_bf[:, ot, qc2 * 2 * QC : (qc2 + 1) * 2 * QC],
                        ps[:, :, :],
                    )

            load_v(0, 4)
            load_kp(4)
            load_v(1, 4)
            load_kp(5)
            load_kp(6)
            load_v(2, 4)
            load_kp(7)
            load_v(3, 4)

            # ---- main loop: emit scores(qc+1) before AV(qc) so ScalarE's
            # ---- exp always has PE runway to hide behind
            def scores(qc):
                p_all = pexp.tile([128, NKT, QC], BF16, tag="p_all")
                for ktg in range(NKT // 2):
                    ps = ps_pool.tile([128, 2, QC], F32, tag="ps")
                    for j in range(2):
                        kt = ktg * 2 + j
                        for it in range(2):
                            nc.tensor.matmul(
                                ps[:, j, :],
                                kT_bf[:, it, kt * 128 : (kt + 1) * 128],
                                wq_bf[:, it, qc * QC : (qc + 1) * QC],
                                start=(it == 0),
                                stop=(it == 1),
                            )
                    nc.scalar.activation(
                        p_all[:, ktg * 2 : (ktg + 1) * 2, :], ps[:, :, :], EXP
                    )
                return p_all

            def av(qc, p_all):
                for qt in range(QC // 128):
                    po = po_pool.tile([128, VN], F32, tag="po")
                    for kt in range(NKT):
                        nc.tensor.matmul(
                            po[:],
                            p_all[:, kt, qt * 128 : (qt + 1) * 128],
                            v_bf[:, kt, :],
                            start=(kt == 0),
                            stop=(kt == NKT - 1),
                        )
                    rec = outp.tile([128, 1], F32, tag="rec")
                    nc.vector.reciprocal(rec[:], po[:, DOUT : DOUT + 1])
                    o_sb = outp.tile([128, DOUT], F32, tag="o_sb")
                    nc.vector.tensor_scalar_mul(o_sb[:], po[:, 0:DOUT], rec[:])
                    r0 = (qc * (QC // 128) + qt) * 128
                    nc.sync.dma_start(o_d.ap()[r0 : r0 + 128, :], o_sb[:])

            p_prev = scores(0)
            for qc in range(1, NQC):
                p_cur = scores(qc)
                av(qc - 1, p_prev)
                p_prev = p_cur
            av(NQC - 1, p_prev)

    nc.compile()
    _prog_cache[repeat] = nc
    return nc


def make_in_maps(q, k, v, w):
    """Shard + marshal full inputs into per-core input maps.

    Marshalling includes the transpose of q/k and the rounding to the
    kernel's compute dtypes (fp16 score path, bf16 values) -- the device
    kernel consumes these layouts directly.
    """
    import ml_dtypes

    q = np.asarray(q, dtype=np.float32)
    k = np.asarray(k, dtype=np.float32)
    v = np.asarray(v, dtype=np.float32)
    w16 = np.ascontiguousarray(np.asarray(w, dtype=np.float32)).astype(np.float16)
    kT = [np.ascontiguousarray(k[b].T).astype(np.float16) for b in range(B)]
    vb = [np.ascontiguousarray(v[b]).astype(ml_dtypes.bfloat16) for b in range(B)]
    in_maps = []
    for c in range(N_CORES):
        b, h = divmod(c, N_CORES // B)
        in_maps.append(
            {
                "qT": np.ascontiguousarray(
                    q[b, h * QS : (h + 1) * QS, :].T
                ).astype(np.float16),
                "kT": kT[b],
                "v": vb[b],
                "w": w16,
            }
        )
    return in_maps


def kernel(q, v, k, w):
    from concourse import bass_utils

    nc = build_program()
    in_maps = make_in_maps(q, k, v, w)
    res = bass_utils.run_bass_kernel_spmd(nc, in_maps, core_ids=list(range(N_CORES)))
    out = np.empty((B, LQ, DOUT), dtype=np.float32)
    for c in range(N_CORES):
        b, h = divmod(c, N_CORES // B)
        out[b, h * QS : (h + 1) * QS, :] = res.results[c]["o"]
    return out
